# revision 20
# baseline (speedup 1.0000x reference)
"""MLA (multi-head latent attention) forward on 8 TRN2 NeuronCores.

Sharding: 8-way tensor-parallel over heads (2 heads/core), both batches
processed sequentially per core. The low-rank A-projections are token-sharded
across all 8 cores (core c computes latents for tokens [256c, 256c+256) of
both batches) and redistributed with pipelined 8-core collectives: one
AllGather per (batch, {q-latent, kv-latent+rope}) plus one AllToAll per batch
that routes each head-pair's rope columns to its owning core. RMSNorm rstd is
folded into the latents before the gather, so every downstream projection
consumes pre-normalized activations and the whole norm pipeline disappears
from the steady state. The collectives run on the TOPSP/SDMA rings and hide
under the previous pipeline stage's matmuls.

Layout: activations are feature-major ([feature, token]); x is transposed
once by the DMA XBAR. Probabilities are computed transposed (s[tk, tq]) so
softmax needs no max-subtraction; denominators come from ones-matmuls + fast
reciprocal. The host sums the 8 per-core o_proj partials per batch.
"""

import numpy as np
import ml_dtypes

B, T, HIDDEN = 2, 2048, 2048
NUM_HEADS = 16
QK_NOPE, QK_ROPE, HEAD_DIM, V_HEAD = 128, 64, 192, 128
KV_LORA, Q_LORA = 512, 1536
EPS = 1e-6
NCORES = 8
HPC = 2   # heads per core
TB = 256  # own tokens per batch (token-shard of the A-projection)

KC = HIDDEN // 128
TT = T // 128
TQ = T // 512
NQ = Q_LORA // 128          # 12 q-latent col tiles
NL = KV_LORA // 128         # 4 kv-latent col tiles
NR = NUM_HEADS * QK_ROPE // 128  # 8 rope col tiles
NKV = NL + NR               # 12 kv col tiles

BF16 = ml_dtypes.bfloat16

_CACHE = {}


def _build():
    import concourse.tile as tile
    from concourse import bacc, mybir
    from concourse.bass import ts

    f32 = mybir.dt.float32
    bf = mybir.dt.bfloat16
    AF = mybir.ActivationFunctionType

    nc = bacc.Bacc(
        "TRN2",
        target_bir_lowering=False,
        debug=False,
        enable_asserts=True,
        num_devices=NCORES,
    )

    def din(name, shape, dt=bf):
        return nc.dram_tensor(name, shape, dt, kind="ExternalInput").ap()

    # weights pre-tiled on host: contiguous per-tile DMA loads
    x_ap = din("x", [2 * TB, HIDDEN])                 # [tok, d] (b0 | b1 slices)
    qaw_ap = din("qaw", [NQ, 128, KC, 128])           # per col-block [p, kk, c]
    kvaw_ap = din("kvaw", [NKV, 128, KC, 128])        # latent 0..3, rope 4..11
    qbw_ap = din("qbw", [128, NQ, 2 * HEAD_DIM])      # [nope0|nope1|ropes]
    kvbw_ap = din("kvbw", [128, NL, HPC * (QK_NOPE + V_HEAD)])
    ow_ap = din("ow", [128, HPC, HIDDEN])
    mask_ap = din("mask", [128, 896])                 # 0/1 causal bank (bf16)
    ones128_ap = din("ones128", [128, 128])
    out_ap = nc.dram_tensor("out", [B, HIDDEN, T], bf, kind="ExternalOutput").ap()

    GROUPS = [list(range(NCORES))]

    # gathered latents (per batch): q [8 ranks x 12 tiles], kv [8 x 12]
    gq_ap = [nc.dram_tensor(f"gq{b}", [NCORES * NQ, 128, TB], bf,
                            kind="Internal", addr_space="Shared").ap()
             for b in range(B)]
    gkv_ap = [nc.dram_tensor(f"gkv{b}", [NCORES * NL, 128, TB], bf,
                             kind="Internal", addr_space="Shared").ap()
              for b in range(B)]
    grope_ap = [nc.dram_tensor(f"grope{b}", [NCORES, 128, TB], bf,
                               kind="Internal").ap()
                for b in range(B)]

    def eng(idx):
        return nc.scalar if idx % 2 else nc.vector

    def copy(e, out, in_):
        if e is nc.scalar:
            nc.scalar.copy(out, in_)
        else:
            nc.vector.tensor_copy(out, in_)

    with tile.TileContext(nc) as tc:
        with tc.tile_pool(name="consts", bufs=1) as consts, \
             tc.tile_pool(name="trans", bufs=3) as trans, \
             tc.tile_pool(name="dram", bufs=1, space="DRAM") as dram, \
             tc.tile_pool(name="act", bufs=1) as act:

            mask = consts.tile([128, 896], bf)
            nc.sync.dma_start(out=mask, in_=mask_ap)
            ones128 = consts.tile([128, 128], bf)
            nc.sync.dma_start(out=ones128, in_=ones128_ap)
            eps1 = consts.tile([128, 1], f32)
            nc.vector.memset(eps1, EPS)

            # ---- Stage A: own-token latents, both batches packed into each
            # 512-wide matmul; collectives issued in downstream-deadline
            # order (q0 | kv0, rope0, q1, kv1, rope1) and pipeline under the
            # remaining compute.
            with tc.tile_pool(name="stageA", bufs=1) as pA, \
                 tc.tile_pool(name="wa", bufs=3) as pwa, \
                 tc.tile_pool(name="pB", bufs=1) as pB, \
                 tc.tile_pool(name="psumA", bufs=1, space="PSUM") as psumA:
                xT = pA.tile([128, KC, 2 * TB], bf)
                for k in range(KC):
                    nc.sync.dma_start(
                        out=xT[:, k, :], in_=x_ap[:, ts(k, 128)], transpose=True
                    )

                latq = pA.tile([128, NQ, 2 * TB], bf)
                latkv = pA.tile([128, NKV, 2 * TB], bf)

                def a_pass(src_ap, ntiles, lat, nsq, lora):
                    psd = None
                    for n in range(ntiles):
                        wa = pwa.tile([128, KC, 128], bf, tag="wa", bufs=3)
                        nc.scalar.dma_start(out=wa, in_=src_ap[n])
                        psm = psumA.tile([128, 2 * TB], f32, tag="psm",
                                         bufs=4)
                        for kk in range(KC):
                            nc.tensor.matmul(
                                out=psm,
                                lhsT=wa[:, kk, :],
                                rhs=xT[:, kk, :],
                                start=(kk == 0),
                                stop=(kk == KC - 1),
                            )
                        nc.scalar.copy(lat[:, n, :], psm)
                        if n < nsq:
                            sq = pB.tile([128, 2 * TB], bf, tag="sq", bufs=4)
                            nc.vector.tensor_mul(sq, lat[:, n, :],
                                                 lat[:, n, :])
                            if n == 0:
                                psd = psumA.tile([128, 2 * TB], f32,
                                                 tag="psd", bufs=2)
                            nc.tensor.matmul(
                                out=psd, lhsT=ones128, rhs=sq,
                                start=(n == 0), stop=(n == nsq - 1),
                            )
                    tmp = pB.tile([128, 2 * TB], f32, tag="tmp", bufs=2)
                    nc.scalar.activation(out=tmp, in_=psd, func=AF.Sqrt,
                                         bias=eps1, scale=1.0 / lora)
                    rr = pB.tile([128, 2 * TB], f32, tag="rr", bufs=2)
                    nc.vector.reciprocal_approx_fast(out=rr, in_=tmp)
                    for n in range(nsq):
                        nc.vector.tensor_mul(lat[:, n, :], lat[:, n, :], rr)

                def ag(gin, gout):
                    nc.gpsimd.collective_compute(
                        "AllGather", mybir.AluOpType.bypass,
                        replica_groups=GROUPS, ins=[gin.opt()], outs=[gout])

                a_pass(qaw_ap, NQ, latq, NQ, Q_LORA)
                ginq = [dram.tile([NQ, 128, TB], bf, name=f"ginq{b}")
                        for b in range(B)]
                for b in range(B):
                    for n in range(NQ):
                        nc.sync.dma_start(out=ginq[b][n],
                                            in_=latq[:, n, ts(b, TB)])
                ag(ginq[0], gq_ap[0])

                a_pass(kvaw_ap, NKV, latkv, NL, KV_LORA)
                ginkv = [dram.tile([NKV, 128, TB], bf, name=f"ginkv{b}")
                         for b in range(B)]
                for b in range(B):
                    for n in range(NKV):
                        nc.sync.dma_start(out=ginkv[b][n],
                                            in_=latkv[:, n, ts(b, TB)])
                ag(ginkv[0][0:NL], gkv_ap[0])
                nc.gpsimd.collective_compute(
                    "AllToAll", mybir.AluOpType.bypass, replica_groups=GROUPS,
                    ins=[ginkv[0][NL:NKV].opt()], outs=[grope_ap[0]])
                ag(ginq[1], gq_ap[1])
                ag(ginkv[1][0:NL], gkv_ap[1])
                nc.gpsimd.collective_compute(
                    "AllToAll", mybir.AluOpType.bypass, replica_groups=GROUPS,
                    ins=[ginkv[1][NL:NKV].opt()], outs=[grope_ap[1]])

            # ---- Stages D/E/F per batch
            xq = act.tile([128, NQ, T], bf)       # reloaded per batch
            xkv = act.tile([128, NL, T], bf)
            krope = [act.tile([128, T], bf, name=f"krope{b}") for b in range(B)]
            qn = act.tile([128, HPC, T], bf)
            qr = act.tile([128, T], bf)           # h0 rope rows 0:64, h1 64:128
            kn = act.tile([128, HPC, T], bf)
            vv = act.tile([128, TT, HPC * V_HEAD], bf)

            with tc.tile_pool(name="wD", bufs=1) as pw, \
                 tc.tile_pool(name="psumD", bufs=1, space="PSUM") as psumD:
                qbw = pw.tile([128, NQ, 2 * HEAD_DIM], bf)
                nc.sync.dma_start(out=qbw, in_=qbw_ap)
                kvbw = pw.tile([128, NL, HPC * (QK_NOPE + V_HEAD)], bf)
                nc.sync.dma_start(out=kvbw, in_=kvbw_ap)
                ow = pw.tile([128, HPC, HIDDEN], bf)
                nc.sync.dma_start(out=ow, in_=ow_ap)
                vcols = kvbw.rearrange(
                    "p kk (h two dv) -> p kk h two dv", h=HPC, two=2
                )

                with tc.tile_pool(name="attn_i", bufs=2) as pai, \
                     tc.tile_pool(name="ob", bufs=2) as pob, \
                     tc.tile_pool(name="rdb", bufs=2) as prdb:

                    ccgate = act.tile([128, TB], bf, name="ccgate")

                    def load_gathered(b):
                        if b == 0:
                            nc.sync.dma_start(out=ccgate, in_=grope_ap[1][0])
                            nc.vector.tensor_copy(ccgate, ccgate)
                        for r in range(NCORES):
                            if b == 0 and r == 0:
                                pass
                            for n in range(NQ):
                                (nc.sync if n % 2 else nc.gpsimd).dma_start(
                                    out=xq[:, n, ts(r, TB)],
                                    in_=gq_ap[b][r * NQ + n])
                            for n in range(NL):
                                (nc.sync if n % 2 else nc.gpsimd).dma_start(
                                    out=xkv[:, n, ts(r, TB)],
                                    in_=gkv_ap[b][r * NL + n])
                            nc.gpsimd.dma_start(
                                out=krope[b][:, ts(r, TB)], in_=grope_ap[b][r])

                    def stage_d(b):
                        # q: nope0 | nope1 | rope pair
                        for t in range(TQ):
                            for sub in range(3):
                                ps = psumD.tile([128, 512], f32, tag="pss",
                                                bufs=4)
                                for kk in range(NQ):
                                    nc.tensor.matmul(
                                        out=ps,
                                        lhsT=qbw[:, kk, ts(sub, 128)],
                                        rhs=xq[:, kk, ts(t, 512)],
                                        start=(kk == 0),
                                        stop=(kk == NQ - 1),
                                    )
                                if sub < 2:
                                    copy(eng(sub), qn[:, sub, ts(t, 512)], ps)
                                else:
                                    nc.scalar.copy(qr[:, ts(t, 512)], ps)
                        for h in range(HPC):
                            for t in range(TQ):
                                ps3 = psumD.tile([128, 512], f32, tag="pss",
                                                 bufs=4)
                                for kk in range(NL):
                                    nc.tensor.matmul(
                                        out=ps3,
                                        lhsT=kvbw[:, kk, ts(h, 256)][:, 0:128],
                                        rhs=xkv[:, kk, ts(t, 512)],
                                        start=(kk == 0),
                                        stop=(kk == NL - 1),
                                    )
                                copy(eng(h + t), kn[:, h, ts(t, 512)], ps3)
                        vvf = vv.rearrange("p a b -> p (a b)")
                        for t2 in range(TT // 2):
                            psv = psumD.tile([128, 512], f32, tag="pss",
                                             bufs=4)
                            for half in range(2):
                                tt = 2 * t2 + half
                                for kk in range(NL):
                                    nc.tensor.matmul(
                                        out=psv[:, ts(half, HPC * V_HEAD)],
                                        lhsT=xkv[:, kk, ts(tt, 128)],
                                        rhs=vcols[:, kk, :, 1, :],
                                        start=(kk == 0),
                                        stop=(kk == NL - 1),
                                    )
                            copy(eng(t2), vvf[:, ts(t2, 512)], psv)

                    attn_tiles = {}

                    def attention_chunk(b, i):
                        # both heads together: the two K=64 rope matmuls land
                        # in disjoint PE row-groups and run concurrently;
                        # denominators via per-head ones-matmul accumulation.
                        attn_i = pai.tile([128, HPC, 512], bf, tag="attn_i",
                                          bufs=2)
                        nj = 4 * i + 4
                        pso = [psumD.tile([128, 512], f32, tag="pso",
                                          bufs=2, name=f"pso{h}")
                               for h in range(HPC)]
                        psd = [psumD.tile([128, 512], f32, tag="psd",
                                          bufs=2, name=f"psd{h}")
                               for h in range(HPC)]

                        def consume_batch(batch, last):
                            for jc, h, exc in batch:
                                nc.tensor.matmul(
                                    out=psd[h], lhsT=ones128, rhs=exc,
                                    start=(jc == 0),
                                    stop=(last and jc == nj - 1),
                                )
                            for jc, h, exc in batch:
                                nc.tensor.matmul(
                                    out=pso[h], lhsT=vv[:, jc, ts(h, V_HEAD)],
                                    rhs=exc,
                                    start=(jc == 0),
                                    stop=(last and jc == nj - 1),
                                )

                        pending = []
                        for j in range(nj):
                            pss = [psumD.tile([128, 512], f32, tag="pss",
                                              bufs=4, name=f"pss{h}")
                                   for h in range(HPC)]
                            for h in range(HPC):
                                nc.tensor.matmul(
                                    out=pss[h],
                                    lhsT=kn[:, h, ts(j, 128)],
                                    rhs=qn[:, h, ts(i, 512)],
                                    start=True,
                                    stop=False,
                                )
                            nc.tensor.matmul(
                                out=pss[0],
                                lhsT=krope[b][0:64, ts(j, 128)],
                                rhs=qr[0:64, ts(i, 512)],
                                start=False,
                                stop=True,
                            )
                            nc.tensor.matmul(
                                out=pss[1],
                                lhsT=krope[b][64:128, ts(j, 128)],
                                rhs=qr[64:128, ts(i, 512)],
                                start=False,
                                stop=True,
                            )
                            if len(pending) >= 4:
                                consume_batch(pending, False)
                                pending = []
                            off = j * 128 - i * 512
                            for h in range(HPC):
                                ex = trans.tile([128, 512], bf, tag="ex",
                                                bufs=8)
                                nc.scalar.activation(out=ex, in_=pss[h],
                                                     func=AF.Exp)
                                if off >= 0:
                                    nc.vector.tensor_mul(
                                        ex, ex, mask[:, 384 - off:896 - off]
                                    )
                                pending.append((j, h, ex))
                        consume_batch(pending, True)

                        for h in range(HPC):
                            rdb = prdb.tile([128, 512], f32, tag="rdb",
                                            bufs=2)
                            nc.vector.reciprocal_approx_fast(out=rdb,
                                                             in_=psd[h])
                            nc.vector.tensor_mul(attn_i[:, h, :], pso[h], rdb)
                        attn_tiles[(b, i)] = attn_i

                    def oproj_chunk(b, i):
                        attn_i = attn_tiles[(b, i)]
                        for m in range(TT):
                            psf = psumD.tile([128, 512], f32, tag="pso",
                                             bufs=2)
                            for kk in range(HPC):
                                nc.tensor.matmul(
                                    out=psf,
                                    lhsT=ow[:, kk, ts(m, 128)],
                                    rhs=attn_i[:, kk, :],
                                    start=(kk == 0),
                                    stop=(kk == HPC - 1),
                                )
                            ob = pob.tile([128, 512], bf, tag="ob", bufs=3)
                            if (b, i) == (B - 1, TQ - 1):
                                copy(eng(m), ob, psf)  # attention done
                            else:
                                nc.vector.tensor_copy(ob, psf)
                            (nc.sync if m % 2 else nc.gpsimd).dma_start(
                                out=out_ap[b, ts(m, 128), ts(i, 512)], in_=ob
                            )

                    load_gathered(0)
                    stage_d(0)
                    load_gathered(1)  # prefetch during b0 attention
                    for b in range(B):
                        if b == 1:
                            stage_d(1)
                        attention_chunk(b, 0)
                        for i in range(1, TQ):
                            attention_chunk(b, i)
                            oproj_chunk(b, i - 1)
                        oproj_chunk(b, TQ - 1)

    nc.compile()
    return nc


def _tile_w(w):
    """[K, N] -> [N/128, 128, K/128, 128] so each col-block loads contiguously."""
    K, N = w.shape
    return np.ascontiguousarray(
        w.reshape(K // 128, 128, N // 128, 128).transpose(2, 1, 0, 3))


def _prep(inputs):
    x = np.asarray(inputs["hidden_states"], np.float32)
    qaw = np.asarray(inputs["q_a_w"], np.float32)
    qalw = np.asarray(inputs["q_a_ln_w"], np.float32)
    qbw = np.asarray(inputs["q_b_w"], np.float32)
    kvaw = np.asarray(inputs["kv_a_w"], np.float32)
    kvlw = np.asarray(inputs["kv_a_ln_w"], np.float32)
    kvbw = np.asarray(inputs["kv_b_w"], np.float32)
    ow = np.asarray(inputs["o_w"], np.float32)

    scale = 1.0 / np.sqrt(np.float32(HEAD_DIM))
    qbw_f = (qbw * qalw[:, None] * scale).astype(BF16)
    kvbw_f = (kvbw * kvlw[:, None]).astype(BF16)
    qaw_t = _tile_w(qaw.astype(BF16))               # [NQ, 128, KC, 128]
    kvaw_t = _tile_w(kvaw.astype(BF16))             # [NKV, 128, KC, 128]

    r = np.arange(128)[:, None]
    j = np.arange(896)[None, :]
    mask = np.where((j - 384) >= r, 1.0, 0.0).astype(BF16)
    ones128 = np.ones((128, 128), BF16)

    def lat_tiled(w):  # [KV_LORA, N] -> [128, NL, N] (p, kk, n)
        return np.ascontiguousarray(w.reshape(NL, 128, -1).transpose(1, 0, 2))

    in_maps = []
    for c in range(NCORES):
        h0 = HPC * c
        qbw_g = qbw_f[:, h0 * HEAD_DIM:(h0 + 2) * HEAD_DIM]
        cols = np.concatenate([
            qbw_g[:, 0 * HEAD_DIM:0 * HEAD_DIM + QK_NOPE],
            qbw_g[:, 1 * HEAD_DIM:1 * HEAD_DIM + QK_NOPE],
            qbw_g[:, 0 * HEAD_DIM + QK_NOPE:1 * HEAD_DIM],
            qbw_g[:, 1 * HEAD_DIM + QK_NOPE:2 * HEAD_DIM],
        ], axis=1)  # [Q_LORA, 384]
        qbw_c = np.ascontiguousarray(
            cols.reshape(NQ, 128, 384).transpose(1, 0, 2))

        sl = slice(TB * c, TB * (c + 1))
        in_maps.append({
            "x": np.concatenate([x[0, sl], x[1, sl]], 0).astype(BF16),
            "qaw": qaw_t,
            "kvaw": kvaw_t,
            "qbw": qbw_c,
            "kvbw": lat_tiled(kvbw_f[:, h0 * 256:(h0 + 2) * 256]),
            "ow": np.ascontiguousarray(
                ow[h0 * V_HEAD:(h0 + 2) * V_HEAD]
                .astype(BF16).reshape(HPC, 128, HIDDEN).transpose(1, 0, 2)),
            "mask": mask,
            "ones128": ones128,
        })
    return in_maps


def _ensure_trace_shim():
    """This image lacks antenv.axon_hooks; synthesize it so a trace=True (or
    BASS_TRACE=1) invocation degrades gracefully instead of crashing."""
    import sys
    import types
    try:
        import antenv.axon_hooks  # noqa: F401
        return
    except Exception:
        pass
    try:
        import antenv
        import trn_agent_boot.trn_boot as tb
        hook = tb._ntff_profile_via_ctypes("/opt/axon/libaxon_pjrt.so")
        mod = types.ModuleType("antenv.axon_hooks")
        mod.get_axon_ntff_profile_hook = lambda: hook
        mod.set_axon_ntff_profile_hook = lambda h: None
        antenv.axon_hooks = mod
        sys.modules["antenv.axon_hooks"] = mod
        import concourse.bass_utils as bu
        bu.upload_artifacts = lambda tmpdir: tmpdir
    except Exception:
        pass


def kernel(**inputs):
    from concourse.bass_utils import run_bass_kernel_spmd

    _ensure_trace_shim()
    if "nc" not in _CACHE:
        _CACHE["nc"] = _build()
    nc = _CACHE["nc"]
    in_maps = _prep(inputs)
    try:
        res = run_bass_kernel_spmd(nc, in_maps, core_ids=list(range(NCORES)),
                                   **_CACHE.get("run_kwargs", {}))
    except Exception:
        # transient accelerator faults (e.g. NRT_EXEC_UNIT_UNRECOVERABLE) have
        # been observed after interrupted runs; one retry clears them
        import time
        time.sleep(2)
        res = run_bass_kernel_spmd(nc, in_maps, core_ids=list(range(NCORES)),
                                   **_CACHE.get("run_kwargs", {}))
    _CACHE["last_results"] = res
    out = np.zeros((B, T, HIDDEN), np.float32)
    for c in range(NCORES):
        r = np.asarray(res.results[c]["out"], np.float32)
        for b in range(B):
            out[b] += r[b].T
    return out


# revision 21
# speedup vs baseline: 1.0428x; 1.0428x over previous
"""MLA (multi-head latent attention) forward on 8 TRN2 NeuronCores.

Sharding: 8-way tensor-parallel over heads (2 heads/core), both batches
processed sequentially per core. The low-rank A-projections are token-sharded
across all 8 cores (core c computes latents for tokens [256c, 256c+256) of
both batches) and redistributed with pipelined 8-core collectives: one
AllGather per (batch, {q-latent, kv-latent+rope}) plus one AllToAll per batch
that routes each head-pair's rope columns to its owning core. RMSNorm rstd is
folded into the latents before the gather, so every downstream projection
consumes pre-normalized activations and the whole norm pipeline disappears
from the steady state. The collectives run on the TOPSP/SDMA rings and hide
under the previous pipeline stage's matmuls.

Layout: activations are feature-major ([feature, token]); x is transposed
once by the DMA XBAR. Probabilities are computed transposed (s[tk, tq]) so
softmax needs no max-subtraction; denominators come from ones-matmuls + fast
reciprocal. The host sums the 8 per-core o_proj partials per batch.
"""

import numpy as np
import ml_dtypes

B, T, HIDDEN = 2, 2048, 2048
NUM_HEADS = 16
QK_NOPE, QK_ROPE, HEAD_DIM, V_HEAD = 128, 64, 192, 128
KV_LORA, Q_LORA = 512, 1536
EPS = 1e-6
NCORES = 8
HPC = 2   # heads per core
TB = 256  # own tokens per batch (token-shard of the A-projection)

KC = HIDDEN // 128
TT = T // 128
TQ = T // 512
NQ = Q_LORA // 128          # 12 q-latent col tiles
NL = KV_LORA // 128         # 4 kv-latent col tiles
NR = NUM_HEADS * QK_ROPE // 128  # 8 rope col tiles
NKV = NL + NR               # 12 kv col tiles

BF16 = ml_dtypes.bfloat16

_CACHE = {}


def _build():
    import concourse.tile as tile
    from concourse import bacc, mybir
    from concourse.bass import ts

    f32 = mybir.dt.float32
    bf = mybir.dt.bfloat16
    AF = mybir.ActivationFunctionType

    nc = bacc.Bacc(
        "TRN2",
        target_bir_lowering=False,
        debug=False,
        enable_asserts=True,
        num_devices=NCORES,
    )

    def din(name, shape, dt=bf):
        return nc.dram_tensor(name, shape, dt, kind="ExternalInput").ap()

    # weights pre-tiled on host: contiguous per-tile DMA loads
    x_ap = din("x", [2 * TB, HIDDEN])                 # [tok, d] (b0 | b1 slices)
    qaw_ap = din("qaw", [NQ, 128, KC, 128])           # per col-block [p, kk, c]
    kvaw_ap = din("kvaw", [NKV, 128, KC, 128])        # latent 0..3, rope 4..11
    qbw_ap = din("qbw", [128, NQ, 2 * HEAD_DIM])      # [nope0|nope1|ropes]
    kvbw_ap = din("kvbw", [128, NL, HPC * (QK_NOPE + V_HEAD)])
    ow_ap = din("ow", [128, HPC, HIDDEN])
    mask_ap = din("mask", [128, 896])                 # 0/1 causal bank (bf16)
    ones128_ap = din("ones128", [128, 128])
    out_ap = nc.dram_tensor("out", [B, HIDDEN, T], bf, kind="ExternalOutput").ap()

    GROUPS = [list(range(NCORES))]

    # gathered latents (per batch): q [8 ranks x 12 tiles], kv [8 x 12]
    gq_ap = [nc.dram_tensor(f"gq{b}", [NCORES * NQ, 128, TB], bf,
                            kind="Internal", addr_space="Shared").ap()
             for b in range(B)]
    gkv_ap = [nc.dram_tensor(f"gkv{b}", [NCORES * NL, 128, TB], bf,
                             kind="Internal", addr_space="Shared").ap()
              for b in range(B)]
    grope_ap = [nc.dram_tensor(f"grope{b}", [NCORES, 128, TB], bf,
                               kind="Internal").ap()
                for b in range(B)]

    def eng(idx):
        return nc.scalar if idx % 2 else nc.vector

    def copy(e, out, in_):
        if e is nc.scalar:
            nc.scalar.copy(out, in_)
        else:
            nc.vector.tensor_copy(out, in_)

    with tile.TileContext(nc) as tc:
        with tc.tile_pool(name="consts", bufs=1) as consts, \
             tc.tile_pool(name="trans", bufs=3) as trans, \
             tc.tile_pool(name="dram", bufs=1, space="DRAM") as dram, \
             tc.tile_pool(name="act", bufs=1) as act:

            mask = consts.tile([128, 896], bf)
            nc.sync.dma_start(out=mask, in_=mask_ap)
            ones128 = consts.tile([128, 128], bf)
            nc.sync.dma_start(out=ones128, in_=ones128_ap)
            eps1 = consts.tile([128, 1], f32)
            nc.vector.memset(eps1, EPS)

            # ---- Stage A: own-token latents, both batches packed into each
            # 512-wide matmul; collectives issued in downstream-deadline
            # order (q0 | kv0, rope0, q1, kv1, rope1) and pipeline under the
            # remaining compute.
            with tc.tile_pool(name="stageA", bufs=1) as pA, \
                 tc.tile_pool(name="wa", bufs=3) as pwa, \
                 tc.tile_pool(name="pB", bufs=1) as pB, \
                 tc.tile_pool(name="psumA", bufs=1, space="PSUM") as psumA:
                xT = pA.tile([128, KC, 2 * TB], bf)
                for k in range(KC):
                    nc.sync.dma_start(
                        out=xT[:, k, :], in_=x_ap[:, ts(k, 128)], transpose=True
                    )

                latq = pA.tile([128, NQ, 2 * TB], bf)
                latkv = pA.tile([128, NKV, 2 * TB], bf)

                def a_pass(src_ap, ntiles, lat, nsq, lora):
                    psd = None
                    for n in range(ntiles):
                        wa = pwa.tile([128, KC, 128], bf, tag="wa", bufs=3)
                        nc.scalar.dma_start(out=wa, in_=src_ap[n])
                        psm = psumA.tile([128, 2 * TB], f32, tag="psm",
                                         bufs=4)
                        for kk in range(KC):
                            nc.tensor.matmul(
                                out=psm,
                                lhsT=wa[:, kk, :],
                                rhs=xT[:, kk, :],
                                start=(kk == 0),
                                stop=(kk == KC - 1),
                            )
                        nc.scalar.copy(lat[:, n, :], psm)
                        if n < nsq:
                            sq = pB.tile([128, 2 * TB], bf, tag="sq", bufs=4)
                            nc.vector.tensor_mul(sq, lat[:, n, :],
                                                 lat[:, n, :])
                            if n == 0:
                                psd = psumA.tile([128, 2 * TB], f32,
                                                 tag="psd", bufs=2)
                            nc.tensor.matmul(
                                out=psd, lhsT=ones128, rhs=sq,
                                start=(n == 0), stop=(n == nsq - 1),
                            )
                    tmp = pB.tile([128, 2 * TB], f32, tag="tmp", bufs=2)
                    nc.scalar.activation(out=tmp, in_=psd, func=AF.Sqrt,
                                         bias=eps1, scale=1.0 / lora)
                    rr = pB.tile([128, 2 * TB], f32, tag="rr", bufs=2)
                    nc.vector.reciprocal_approx_fast(out=rr, in_=tmp)
                    for n in range(nsq):
                        nc.vector.tensor_mul(lat[:, n, :], lat[:, n, :], rr)

                def ag(gin, gout):
                    nc.gpsimd.collective_compute(
                        "AllGather", mybir.AluOpType.bypass,
                        replica_groups=GROUPS, ins=[gin.opt()], outs=[gout])

                a_pass(qaw_ap, NQ, latq, NQ, Q_LORA)
                ginq = [dram.tile([NQ, 128, TB], bf, name=f"ginq{b}")
                        for b in range(B)]
                for b in range(B):
                    for n in range(NQ):
                        nc.sync.dma_start(out=ginq[b][n],
                                            in_=latq[:, n, ts(b, TB)])
                ag(ginq[0], gq_ap[0])

                a_pass(kvaw_ap, NKV, latkv, NL, KV_LORA)
                ginkv = [dram.tile([NKV, 128, TB], bf, name=f"ginkv{b}")
                         for b in range(B)]
                for b in range(B):
                    for n in range(NKV):
                        nc.sync.dma_start(out=ginkv[b][n],
                                            in_=latkv[:, n, ts(b, TB)])
                ag(ginkv[0][0:NL], gkv_ap[0])
                nc.gpsimd.collective_compute(
                    "AllToAll", mybir.AluOpType.bypass, replica_groups=GROUPS,
                    ins=[ginkv[0][NL:NKV].opt()], outs=[grope_ap[0]])
                ag(ginq[1], gq_ap[1])
                ag(ginkv[1][0:NL], gkv_ap[1])
                nc.gpsimd.collective_compute(
                    "AllToAll", mybir.AluOpType.bypass, replica_groups=GROUPS,
                    ins=[ginkv[1][NL:NKV].opt()], outs=[grope_ap[1]])

            # ---- Stages D/E/F per batch
            xq = act.tile([128, NQ, T], bf)       # reloaded per batch
            xkv = act.tile([128, NL, T], bf)
            krope = [act.tile([128, T], bf, name=f"krope{b}") for b in range(B)]
            qn = act.tile([128, HPC, T], bf)
            qr = act.tile([128, T], bf)           # h0 rope rows 0:64, h1 64:128
            kn = act.tile([128, HPC, T], bf)
            vv = act.tile([128, TT, HPC * V_HEAD], bf)

            with tc.tile_pool(name="wD", bufs=1) as pw, \
                 tc.tile_pool(name="psumD", bufs=1, space="PSUM") as psumD:
                qbw = pw.tile([128, NQ, 2 * HEAD_DIM], bf)
                nc.sync.dma_start(out=qbw, in_=qbw_ap)
                kvbw = pw.tile([128, NL, HPC * (QK_NOPE + V_HEAD)], bf)
                nc.sync.dma_start(out=kvbw, in_=kvbw_ap)
                ow = pw.tile([128, HPC, HIDDEN], bf)
                nc.sync.dma_start(out=ow, in_=ow_ap)
                vcols = kvbw.rearrange(
                    "p kk (h two dv) -> p kk h two dv", h=HPC, two=2
                )

                with tc.tile_pool(name="attn_i", bufs=2) as pai, \
                     tc.tile_pool(name="ob", bufs=2) as pob, \
                     tc.tile_pool(name="rdb", bufs=2) as prdb:

                    def load_gathered(b):
                        for r in range(NCORES):
                            for n in range(NQ):
                                (nc.sync if n % 2 else nc.gpsimd).dma_start(
                                    out=xq[:, n, ts(r, TB)],
                                    in_=gq_ap[b][r * NQ + n])
                            for n in range(NL):
                                (nc.sync if n % 2 else nc.gpsimd).dma_start(
                                    out=xkv[:, n, ts(r, TB)],
                                    in_=gkv_ap[b][r * NL + n])
                            nc.gpsimd.dma_start(
                                out=krope[b][:, ts(r, TB)], in_=grope_ap[b][r])

                    def stage_d(b):
                        # q: nope0 | nope1 | rope pair
                        for t in range(TQ):
                            for sub in range(3):
                                ps = psumD.tile([128, 512], f32, tag="pss",
                                                bufs=4)
                                for kk in range(NQ):
                                    nc.tensor.matmul(
                                        out=ps,
                                        lhsT=qbw[:, kk, ts(sub, 128)],
                                        rhs=xq[:, kk, ts(t, 512)],
                                        start=(kk == 0),
                                        stop=(kk == NQ - 1),
                                    )
                                if sub < 2:
                                    copy(eng(sub), qn[:, sub, ts(t, 512)], ps)
                                else:
                                    nc.scalar.copy(qr[:, ts(t, 512)], ps)
                        for h in range(HPC):
                            for t in range(TQ):
                                ps3 = psumD.tile([128, 512], f32, tag="pss",
                                                 bufs=4)
                                for kk in range(NL):
                                    nc.tensor.matmul(
                                        out=ps3,
                                        lhsT=kvbw[:, kk, ts(h, 256)][:, 0:128],
                                        rhs=xkv[:, kk, ts(t, 512)],
                                        start=(kk == 0),
                                        stop=(kk == NL - 1),
                                    )
                                copy(eng(h + t), kn[:, h, ts(t, 512)], ps3)
                        vvf = vv.rearrange("p a b -> p (a b)")
                        for t2 in range(TT // 2):
                            psv = psumD.tile([128, 512], f32, tag="pss",
                                             bufs=4)
                            for half in range(2):
                                tt = 2 * t2 + half
                                for kk in range(NL):
                                    nc.tensor.matmul(
                                        out=psv[:, ts(half, HPC * V_HEAD)],
                                        lhsT=xkv[:, kk, ts(tt, 128)],
                                        rhs=vcols[:, kk, :, 1, :],
                                        start=(kk == 0),
                                        stop=(kk == NL - 1),
                                    )
                            copy(eng(t2), vvf[:, ts(t2, 512)], psv)

                    attn_tiles = {}

                    def attention_chunk(b, i):
                        # both heads together: the two K=64 rope matmuls land
                        # in disjoint PE row-groups and run concurrently;
                        # denominators via per-head ones-matmul accumulation.
                        attn_i = pai.tile([128, HPC, 512], bf, tag="attn_i",
                                          bufs=2)
                        nj = 4 * i + 4
                        pso = [psumD.tile([128, 512], f32, tag="pso",
                                          bufs=2, name=f"pso{h}")
                               for h in range(HPC)]
                        psd = [psumD.tile([128, 512], f32, tag="psd",
                                          bufs=2, name=f"psd{h}")
                               for h in range(HPC)]

                        def consume_batch(batch, last):
                            for jc, h, exc in batch:
                                nc.tensor.matmul(
                                    out=psd[h], lhsT=ones128, rhs=exc,
                                    start=(jc == 0),
                                    stop=(last and jc == nj - 1),
                                )
                            for jc, h, exc in batch:
                                nc.tensor.matmul(
                                    out=pso[h], lhsT=vv[:, jc, ts(h, V_HEAD)],
                                    rhs=exc,
                                    start=(jc == 0),
                                    stop=(last and jc == nj - 1),
                                )

                        pending = []
                        for j in range(nj):
                            pss = [psumD.tile([128, 512], f32, tag="pss",
                                              bufs=4, name=f"pss{h}")
                                   for h in range(HPC)]
                            for h in range(HPC):
                                nc.tensor.matmul(
                                    out=pss[h],
                                    lhsT=kn[:, h, ts(j, 128)],
                                    rhs=qn[:, h, ts(i, 512)],
                                    start=True,
                                    stop=False,
                                )
                            nc.tensor.matmul(
                                out=pss[0],
                                lhsT=krope[b][0:64, ts(j, 128)],
                                rhs=qr[0:64, ts(i, 512)],
                                start=False,
                                stop=True,
                            )
                            nc.tensor.matmul(
                                out=pss[1],
                                lhsT=krope[b][64:128, ts(j, 128)],
                                rhs=qr[64:128, ts(i, 512)],
                                start=False,
                                stop=True,
                            )
                            if len(pending) >= 4:
                                consume_batch(pending, False)
                                pending = []
                            off = j * 128 - i * 512
                            for h in range(HPC):
                                ex = trans.tile([128, 512], bf, tag="ex",
                                                bufs=8)
                                nc.scalar.activation(out=ex, in_=pss[h],
                                                     func=AF.Exp)
                                if off >= 0:
                                    nc.vector.tensor_mul(
                                        ex, ex, mask[:, 384 - off:896 - off]
                                    )
                                pending.append((j, h, ex))
                        consume_batch(pending, True)

                        for h in range(HPC):
                            rdb = prdb.tile([128, 512], f32, tag="rdb",
                                            bufs=2)
                            nc.vector.reciprocal_approx_fast(out=rdb,
                                                             in_=psd[h])
                            nc.vector.tensor_mul(attn_i[:, h, :], pso[h], rdb)
                        attn_tiles[(b, i)] = attn_i

                    def oproj_chunk(b, i):
                        attn_i = attn_tiles[(b, i)]
                        for m in range(TT):
                            psf = psumD.tile([128, 512], f32, tag="pso",
                                             bufs=2)
                            for kk in range(HPC):
                                nc.tensor.matmul(
                                    out=psf,
                                    lhsT=ow[:, kk, ts(m, 128)],
                                    rhs=attn_i[:, kk, :],
                                    start=(kk == 0),
                                    stop=(kk == HPC - 1),
                                )
                            ob = pob.tile([128, 512], bf, tag="ob", bufs=3)
                            if (b, i) == (B - 1, TQ - 1):
                                copy(eng(m), ob, psf)  # attention done
                            else:
                                nc.vector.tensor_copy(ob, psf)
                            (nc.sync if m % 2 else nc.gpsimd).dma_start(
                                out=out_ap[b, ts(m, 128), ts(i, 512)], in_=ob
                            )

                    load_gathered(0)
                    stage_d(0)
                    load_gathered(1)  # prefetch during b0 attention
                    for b in range(B):
                        if b == 1:
                            stage_d(1)
                        attention_chunk(b, 0)
                        for i in range(1, TQ):
                            attention_chunk(b, i)
                            oproj_chunk(b, i - 1)
                        oproj_chunk(b, TQ - 1)

    nc.compile()
    return nc


def _tile_w(w):
    """[K, N] -> [N/128, 128, K/128, 128] so each col-block loads contiguously."""
    K, N = w.shape
    return np.ascontiguousarray(
        w.reshape(K // 128, 128, N // 128, 128).transpose(2, 1, 0, 3))


def _prep(inputs):
    x = np.asarray(inputs["hidden_states"], np.float32)
    qaw = np.asarray(inputs["q_a_w"], np.float32)
    qalw = np.asarray(inputs["q_a_ln_w"], np.float32)
    qbw = np.asarray(inputs["q_b_w"], np.float32)
    kvaw = np.asarray(inputs["kv_a_w"], np.float32)
    kvlw = np.asarray(inputs["kv_a_ln_w"], np.float32)
    kvbw = np.asarray(inputs["kv_b_w"], np.float32)
    ow = np.asarray(inputs["o_w"], np.float32)

    scale = 1.0 / np.sqrt(np.float32(HEAD_DIM))
    qbw_f = (qbw * qalw[:, None] * scale).astype(BF16)
    kvbw_f = (kvbw * kvlw[:, None]).astype(BF16)
    qaw_t = _tile_w(qaw.astype(BF16))               # [NQ, 128, KC, 128]
    kvaw_t = _tile_w(kvaw.astype(BF16))             # [NKV, 128, KC, 128]

    r = np.arange(128)[:, None]
    j = np.arange(896)[None, :]
    mask = np.where((j - 384) >= r, 1.0, 0.0).astype(BF16)
    ones128 = np.ones((128, 128), BF16)

    def lat_tiled(w):  # [KV_LORA, N] -> [128, NL, N] (p, kk, n)
        return np.ascontiguousarray(w.reshape(NL, 128, -1).transpose(1, 0, 2))

    in_maps = []
    for c in range(NCORES):
        h0 = HPC * c
        qbw_g = qbw_f[:, h0 * HEAD_DIM:(h0 + 2) * HEAD_DIM]
        cols = np.concatenate([
            qbw_g[:, 0 * HEAD_DIM:0 * HEAD_DIM + QK_NOPE],
            qbw_g[:, 1 * HEAD_DIM:1 * HEAD_DIM + QK_NOPE],
            qbw_g[:, 0 * HEAD_DIM + QK_NOPE:1 * HEAD_DIM],
            qbw_g[:, 1 * HEAD_DIM + QK_NOPE:2 * HEAD_DIM],
        ], axis=1)  # [Q_LORA, 384]
        qbw_c = np.ascontiguousarray(
            cols.reshape(NQ, 128, 384).transpose(1, 0, 2))

        sl = slice(TB * c, TB * (c + 1))
        in_maps.append({
            "x": np.concatenate([x[0, sl], x[1, sl]], 0).astype(BF16),
            "qaw": qaw_t,
            "kvaw": kvaw_t,
            "qbw": qbw_c,
            "kvbw": lat_tiled(kvbw_f[:, h0 * 256:(h0 + 2) * 256]),
            "ow": np.ascontiguousarray(
                ow[h0 * V_HEAD:(h0 + 2) * V_HEAD]
                .astype(BF16).reshape(HPC, 128, HIDDEN).transpose(1, 0, 2)),
            "mask": mask,
            "ones128": ones128,
        })
    return in_maps


def _ensure_trace_shim():
    """This image lacks antenv.axon_hooks; synthesize it so a trace=True (or
    BASS_TRACE=1) invocation degrades gracefully instead of crashing."""
    import sys
    import types
    try:
        import antenv.axon_hooks  # noqa: F401
        return
    except Exception:
        pass
    try:
        import antenv
        import trn_agent_boot.trn_boot as tb
        hook = tb._ntff_profile_via_ctypes("/opt/axon/libaxon_pjrt.so")
        mod = types.ModuleType("antenv.axon_hooks")
        mod.get_axon_ntff_profile_hook = lambda: hook
        mod.set_axon_ntff_profile_hook = lambda h: None
        antenv.axon_hooks = mod
        sys.modules["antenv.axon_hooks"] = mod
        import concourse.bass_utils as bu
        bu.upload_artifacts = lambda tmpdir: tmpdir
    except Exception:
        pass


def kernel(**inputs):
    from concourse.bass_utils import run_bass_kernel_spmd

    _ensure_trace_shim()
    if "nc" not in _CACHE:
        _CACHE["nc"] = _build()
    nc = _CACHE["nc"]
    in_maps = _prep(inputs)
    try:
        res = run_bass_kernel_spmd(nc, in_maps, core_ids=list(range(NCORES)),
                                   **_CACHE.get("run_kwargs", {}))
    except Exception:
        # transient accelerator faults (e.g. NRT_EXEC_UNIT_UNRECOVERABLE) have
        # been observed after interrupted runs; one retry clears them
        import time
        time.sleep(2)
        res = run_bass_kernel_spmd(nc, in_maps, core_ids=list(range(NCORES)),
                                   **_CACHE.get("run_kwargs", {}))
    _CACHE["last_results"] = res
    out = np.zeros((B, T, HIDDEN), np.float32)
    for c in range(NCORES):
        r = np.asarray(res.results[c]["out"], np.float32)
        for b in range(B):
            out[b] += r[b].T
    return out
